# revision 5
# baseline (speedup 1.0000x reference)
"""Trainium2 Bass kernel v7: Gaussian-splat covariance from (scaling, rotation).

Math (per point): s = sigmoid(sc)*(SMAX-SMIN)+SMIN; q normalized quaternion;
R = rot matrix; C = R diag(s^2) R^T; out = upper-tri-6 of C.

Rank-2 reformulation (the 3rd column of R is never materialized):
  C = s2z*I + (dxr*iv4)*(A A^T) + (dyr*iv4)*(B B^T)
  A = a/2, B = b/2 (a,b = first two unnormalized columns of R*n2)
  n2' = n2/2 (from ACT Square with scale 2^-1/2); iv4 = n2'^-2 = 4/n2^2
  A0 = p-q, A1 = xy+rz, A2 = xz-ry; B0 = xy-rz, B1 = pm+qm, B2 = yz+rx
  iv4 via ACT tables: exp(-2*ln(n2')).

All bf16 except n2' (f32 Ln input) and the f32 outputs. Validated vs f64
reference: rel ~5.9e-3 (tolerance 2e-2).

Layout: bf16 planes of f points per partition; HBM I/O stays interleaved
(contiguous DMA); de/interleave happens inside compute-op access patterns.
First/last tiles are split 4x smaller to shorten pipeline fill/drain.

Engine split: ACT = quaternion deinterleave cast + squares + sigmoid +
ln/exp + offdiag output casts; DVE = packed-bf16 2x tensor_tensor chain;
Pool = dxr/dyr + diag output adds. Output stage runs one tile behind
compute (software pipeline).
"""

import numpy as np

import concourse.bass as bass
import concourse.mybir as mybir
from concourse.tile import TileContext

F32 = mybir.dt.float32
BF16 = mybir.dt.bfloat16
ALU = mybir.AluOpType
ACTF = mybir.ActivationFunctionType

SCALE_MIN = 1e-4
SCALE_MAX = 10.0
A_SC = SCALE_MAX - SCALE_MIN
B_SC = SCALE_MIN

N_CORES = 8
N_TOTAL = 4_000_000

F_PTS = 784
T_TILES = 5
P_CORE = 128 * F_PTS * T_TILES  # 501760; 8 cores cover 4,014,080 >= 4e6


def _pl(tile_ap, k, f, i, n=1, step=1):
    """Planar tile view [P, k*f] -> [P, n, f]: planes i, i+step, ..."""
    r = tile_ap[:, : k * f].rearrange("p (k f) -> p k f", k=k)
    if step == 1:
        return r[:, i : i + n]
    if step < 0:
        lo = i + (n - 1) * step
        return r[:, i : (lo - 1 if lo > 0 else None) : step]
    return r[:, i : i + (n - 1) * step + 1 : step]


def _plb(tile_ap, k, f, i, n):
    """Broadcast plane i across n mid-lanes -> [P, n, f]."""
    r = tile_ap[:, : k * f].rearrange("p (k f) -> p k f", k=k)
    one = r[:, i : i + 1]
    return one.broadcast_to((one.shape[0], n, one.shape[2]))


def _split_sync_waits(nc, nop_max=1):
    """This container's walrus encodes at most 2 sync waits per instruction
    (and none on Drain). Move excess waits onto dedicated NoOps upstream."""
    n = 0
    for bb in nc.main_func.blocks:
        out = []
        for ins in bb.instructions:
            si = ins.sync_info
            waits = list(si.on_wait) if (si is not None and si.on_wait) else []
            is_drain = type(ins).__name__ == "InstDrain"
            limit = 0 if is_drain and len(waits) > 1 else 1
            if len(waits) > limit:
                keep = waits[-limit:] if limit else []
                extra = waits[:-limit] if limit else waits
                for i0 in range(0, len(extra), nop_max):
                    n += 1
                    nop = mybir.InstNoOp(name=f"waitsplit_{n}", ins=[], outs=[])
                    nop.engine = ins.engine
                    nop.sync_info = mybir.SyncInfo(
                        on_wait=extra[i0 : i0 + nop_max], on_update=[]
                    )
                    out.append(nop)
                ins.sync_info = mybir.SyncInfo(
                    on_wait=keep, on_update=list(si.on_update or [])
                )
            out.append(ins)
        bb.instructions[:] = out
    return n


def build_nc(F=F_PTS, T=T_TILES, split_waits=True, split_edge=4):
    """Build the per-core Bass program. Same program on all 8 cores."""
    nc = bass.Bass()
    P = 128
    npts = P * F * T

    _bconst = nc.alloc_sbuf_tensor("const-f32-bsc", [P, 1], F32)
    nc.gpsimd.memset(_bconst.ap(), B_SC)
    nc.const_aps.aps[(F32, B_SC)] = _bconst.ap()
    nc.all_engine_barrier()

    rot_d = nc.declare_dram_parameter("rotation", [npts, 4], F32, isOutput=False)
    scal_d = nc.declare_dram_parameter("scaling", [npts, 3], F32, isOutput=False)
    out_d = nc.declare_dram_parameter("symm", [npts, 6], F32, isOutput=True)

    ve = nc.vector
    act = nc.scalar
    po = nc.gpsimd

    # segment list: (row_start, f); first/last tile split to shorten
    # pipeline fill and drain
    segs = []
    for t in range(T):
        base = t * P * F
        if t in (0, T - 1) and split_edge > 1:
            q = F // split_edge
            segs += [(base + i * P * q, q) for i in range(split_edge)]
        else:
            segs.append((base, F))

    def emit_output_stage(C6, S2I, OUT, rows, f):
        """bf16 C6 planes + s2z -> interleaved f32 OUT, then DMA. Runs one
        segment behind compute. Plane-major iteration = unit-stride reads."""
        outk = OUT[:, : 6 * f].rearrange("p (f c) -> p c f", c=6)
        c6k = C6[:, : 6 * f].rearrange("p (k f) -> p k f", k=6)
        s2zk = S2I[:, : 3 * f].rearrange("p (f c) -> p c f", c=3)[:, 2:3]
        po.tensor_tensor(
            outk[:, 0:4:3], c6k[:, 0:4:3],
            s2zk.broadcast_to((P, 2, f)), ALU.add,
        )
        po.tensor_tensor(outk[:, 5:6], c6k[:, 5:6], s2zk, ALU.add)
        act.copy(outk[:, 1:3], c6k[:, 1:3])
        act.copy(outk[:, 4:5], c6k[:, 4:5])
        nc.sync.dma_start(
            out_d[rows, :].rearrange("(p f) c -> p (f c)", p=P), OUT[:, : 6 * f]
        )

    with TileContext(nc) as tc:
        with (
            tc.tile_pool(name="io", bufs=2) as io,
            tc.tile_pool(name="acto", bufs=2) as acto,
            tc.tile_pool(name="s2ip", bufs=4) as s2ip,
            tc.tile_pool(name="c6p", bufs=3) as c6p,
            tc.tile_pool(name="work", bufs=1) as work,
        ):
            prev = None
            for row0, f in segs:
                rows = slice(row0, row0 + P * f)

                ROT = io.tile([P, 4 * f], F32, tag="rot")
                SCAL = io.tile([P, 3 * f], F32, tag="scal")
                OUT = io.tile([P, 6 * f], F32, tag="out")
                nc.sync.dma_start(
                    ROT[:], rot_d[rows, :].rearrange("(p f) c -> p (f c)", p=P)
                )
                nc.sync.dma_start(
                    SCAL[:], scal_d[rows, :].rearrange("(p f) c -> p (f c)", p=P)
                )

                SQP = acto.tile([P, 4 * f], BF16, tag="sqp")  # hr hx hy hz
                SGI = acto.tile([P, 3 * f], BF16, tag="sgi")
                QP = acto.tile([P, 4 * f], BF16, tag="qp")  # r x y z planes
                S2I = s2ip.tile([P, 3 * f], BF16, tag="s2i")
                PQ = work.tile([P, 2 * f], BF16, tag="pq")  # p q
                PM = work.tile([P, 2 * f], BF16, tag="pm")  # pm qm -> later DD
                N2 = work.tile([P, f], mybir.dt.float16, tag="n2")
                LNN = work.tile([P, f], BF16, tag="lnn")
                IV4 = work.tile([P, f], BF16, tag="iv4")
                PRD = work.tile([P, 6 * f], BF16, tag="prd")  # xy xz ry rz rx yz
                ABT = work.tile([P, 6 * f], BF16, tag="abt")  # A0 A1 A2 B0 B1 B2
                TU = work.tile([P, 12 * f], BF16, tag="tu")
                C6 = c6p.tile([P, 6 * f], BF16, tag="c6")

                rot_perm = ROT[:, : 4 * f].rearrange("p (f c) -> p c f", c=4)

                # --- deinterleave quaternion: split ACT (r,x) / DVE (y,z)
                # to balance engine load ---
                act.copy(_pl(QP, 4, f, 0, 2), rot_perm[:, 0:2])
                ve.tensor_copy(_pl(QP, 4, f, 2, 2), rot_perm[:, 2:4])
                act.activation(
                    _pl(SQP, 4, f, 0, 4), _pl(QP, 4, f, 0, 4), ACTF.Square,
                    scale=2**-0.5,
                )
                act.activation(SGI[:], SCAL[:], ACTF.Sigmoid)
                act.activation(S2I[:], SGI[:], ACTF.Square, bias=B_SC, scale=A_SC)

                # --- DVE: butterflies ---
                ve.tensor_tensor(
                    _pl(PQ, 2, f, 0, 2), _pl(SQP, 4, f, 0, 2, 2),
                    _pl(SQP, 4, f, 1, 2, 2), ALU.add,
                )
                ve.tensor_tensor(
                    _pl(PM, 2, f, 0, 2), _pl(SQP, 4, f, 0, 2, 2),
                    _pl(SQP, 4, f, 1, 2, 2), ALU.subtract,
                )
                ve.tensor_tensor(
                    N2[:].unsqueeze(1), _pl(PQ, 2, f, 0), _pl(PQ, 2, f, 1),
                    ALU.add,
                )
                if f < F:
                    # edge segments: DVE reciprocal + switch-free ACT square.
                    # The Ln/Exp round-trip (incl 1.28us table switch)
                    # exceeds the small segment's Vector work and left a
                    # measured ~1.75us Vector idle gap per edge segment.
                    IV2 = work.tile([P, f], F32, tag="iv2")
                    ve.reciprocal(IV2[:], N2[:])
                    act.activation(IV4[:], IV2[:], ACTF.Square)
                else:
                    act.activation(LNN[:], N2[:], ACTF.Ln)
                    act.activation(IV4[:], LNN[:], ACTF.Exp, scale=-2.0)

                ve.tensor_tensor(
                    _pl(ABT, 6, f, 0), _pl(PQ, 2, f, 0), _pl(PQ, 2, f, 1),
                    ALU.subtract,
                )
                ve.tensor_tensor(
                    _pl(ABT, 6, f, 4), _pl(PM, 2, f, 0), _pl(PM, 2, f, 1),
                    ALU.add,
                )

                # --- DVE: products (xy,xz,ry,rz) fused + (rx,yz) ---
                prd4 = PRD[:, : 4 * f].rearrange("p (a b f) -> p a b f", a=2, b=2)
                xr = _pl(QP, 4, f, 1, 2, -1).unsqueeze(2).broadcast_to((P, 2, 2, f))
                yz2 = _pl(QP, 4, f, 2, 2).unsqueeze(1).broadcast_to((P, 2, 2, f))
                ve.tensor_tensor(prd4, xr, yz2, ALU.mult)
                ve.tensor_tensor(
                    _pl(PRD, 6, f, 4, 2), _pl(QP, 4, f, 0, 2, 2),
                    _pl(QP, 4, f, 1, 2, 2), ALU.mult,
                )
                # (A1,B2) = (xy,yz)+(rz,rx) -> ABT planes (1,5)
                ve.tensor_tensor(
                    _pl(ABT, 6, f, 1, 2, 4), _pl(PRD, 6, f, 0, 2, 5),
                    _pl(PRD, 6, f, 3, 2, 1), ALU.add,
                )
                # (B0,A2) = (xy,xz)-(rz,ry) -> ABT planes (3,2)
                ve.tensor_tensor(
                    _pl(ABT, 6, f, 3, 2, -1), _pl(PRD, 6, f, 0, 2, 1),
                    _pl(PRD, 6, f, 3, 2, -1), ALU.subtract,
                )

                # --- Pool: dxr/dyr ---
                s2i_cv = S2I[:, : 3 * f].rearrange("p (f c) -> p c f", c=3)
                po.tensor_tensor(
                    _pl(PQ, 2, f, 0, 2), s2i_cv[:, 0:2],
                    s2i_cv[:, 2:3].broadcast_to((P, 2, f)), ALU.subtract,
                )
                # DD -> overwrites PM (dead after B1); stays on DVE
                ve.tensor_tensor(
                    _pl(PM, 2, f, 0, 2), _pl(PQ, 2, f, 0, 2),
                    IV4[:].unsqueeze(1).broadcast_to((P, 2, f)), ALU.mult,
                )

                # --- DVE: PAB = ABT * DD (into PRD, dead after combines) ---
                abt_g = ABT[:, : 6 * f].rearrange("p (g c f) -> p g c f", g=2, c=3)
                pab_g = PRD[:, : 6 * f].rearrange("p (g c f) -> p g c f", g=2, c=3)
                dd_b = (
                    PM[:, : 2 * f].rearrange("p (g f) -> p g f", g=2)
                    .unsqueeze(2)
                    .broadcast_to((P, 2, 3, f))
                )
                ve.tensor_tensor(pab_g, abt_g, dd_b, ALU.mult)

                # --- DVE: TU pairs t_ik = PAB_i * ABT_k ---
                tu_g = TU[:, : 12 * f].rearrange("p (g k f) -> p g k f", g=2, k=6)
                ve.tensor_tensor(
                    tu_g[:, :, 0:3],
                    pab_g[:, :, 0:1].broadcast_to((P, 2, 3, f)),
                    abt_g, ALU.mult,
                )
                ve.tensor_tensor(
                    tu_g[:, :, 3:5],
                    pab_g[:, :, 1:2].broadcast_to((P, 2, 2, f)),
                    abt_g[:, :, 1:3], ALU.mult,
                )
                ve.tensor_tensor(
                    tu_g[:, :, 5:6], pab_g[:, :, 2:3], abt_g[:, :, 2:3],
                    ALU.mult,
                )

                # --- DVE: C6 = t + u ---
                ve.tensor_tensor(
                    C6[:, : 6 * f], TU[:, 0 : 6 * f], TU[:, 6 * f : 12 * f],
                    ALU.add,
                )

                if prev is not None:
                    emit_output_stage(*prev)
                prev = (C6, S2I, OUT, rows, f)
            emit_output_stage(*prev)
    if split_waits:
        _split_sync_waits(nc)
    return nc


_NC_CACHE = {}


def _get_nc(F, T):
    key = (F, T)
    if key not in _NC_CACHE:
        _NC_CACHE[key] = build_nc(F, T)
    return _NC_CACHE[key]


P = 128


def kernel(scaling: np.ndarray, rotation: np.ndarray) -> np.ndarray:
    from concourse.bass_utils import run_bass_kernel_spmd

    scaling = np.ascontiguousarray(np.asarray(scaling, dtype=np.float32))
    rotation = np.ascontiguousarray(np.asarray(rotation, dtype=np.float32))
    n = scaling.shape[0]

    ntot = N_CORES * P_CORE
    scal_p = np.zeros((ntot, 3), dtype=np.float32)
    rot_p = np.zeros((ntot, 4), dtype=np.float32)
    rot_p[:, 0] = 1.0  # benign quaternion for padding
    scal_p[:n] = scaling
    rot_p[:n] = rotation

    nc = _get_nc(F_PTS, T_TILES)
    in_maps = [
        {
            "scaling": scal_p[i * P_CORE : (i + 1) * P_CORE],
            "rotation": rot_p[i * P_CORE : (i + 1) * P_CORE],
        }
        for i in range(N_CORES)
    ]
    res = run_bass_kernel_spmd(nc, in_maps, list(range(N_CORES)))
    out = np.concatenate([res.results[i]["symm"] for i in range(N_CORES)], axis=0)
    return out[:n]
